# revision 10
# baseline (speedup 1.0000x reference)
"""Trainium2 Bass kernel for MambaMomentum (B=1, L=2048, D=1024, ED=2048, N=16).

Strategy: tensor-parallel over d_inner (ED) across 8 NeuronCores.
Each core owns 256 ED channels: in_proj columns, depthwise conv, the
(ED x N) selective scan with momentum, and out_proj rows. The only
cross-core dependency is dBC = xc @ W_x.T (a full-ED contraction),
handled with one small on-device AllReduce (2048 x 96 fp32). out_proj
partial products are summed on the host (partial-sum unshard).

Scan mapping: e-channels on partitions, time on the free dimension;
DVE TensorTensorScan computes h_t = a_t * h_{t-1} + v_t directly.
The momentum recurrence v_t = beta*v_{t-1} + u_t is the same scan with
a constant decay.
"""

import sys

if "/opt/trn_rl_repo" not in sys.path:
    sys.path.insert(0, "/opt/trn_rl_repo")

import numpy as np
import ml_dtypes

import concourse.bass as bass
import concourse.mybir as mybir
from concourse.tile import TileContext

N_CORES = 8
D_MODEL = 1024
ED = 2048
N_ST = 16          # d_state
DT_RANK = 64
K_CONV = 4
BETA = 0.6
ALPHA = 1.0
L = 2048
E = ED // N_CORES  # 256 channels per core
NE = E // 128      # 2 e-tiles per core
NT = L // 512      # 4 t-chunks
DBC = DT_RANK + 2 * N_ST  # 96
BF16 = mybir.dt.bfloat16
F32 = mybir.dt.float32
F32R = mybir.dt.float32r
AF = mybir.ActivationFunctionType
OP = mybir.AluOpType

_CACHE = {}


def _split_ctrl_waits(nc, max_waits=1):
    """walrus CoreV3 codegen rejects instructions with >1 sem-wait on several
    encodings; move excess waits onto single-wait NoOps just before."""
    for fn in nc.m.functions:
        for bb in fn.blocks:
            new_insts = []
            for inst in bb.instructions:
                si = inst.sync_info
                if si is not None and si.on_wait and len(si.on_wait) > max_waits:
                    waits = list(si.on_wait)
                    si.on_wait = waits[:max_waits]
                    extra = waits[max_waits:]
                    for i in range(0, len(extra), max_waits):
                        new_insts.append(mybir.InstNoOp(
                            name=f"{inst.name}_ws{i}",
                            engine=inst.engine,
                            ins=[], outs=[],
                            sync_info=mybir.SyncInfo(
                                on_wait=extra[i:i + max_waits], on_update=[]),
                        ))
                new_insts.append(inst)
            bb.instructions[:] = new_insts


def _build_program():
    nc = bass.Bass("TRN2", target_bir_lowering=False, debug=False,
                   num_devices=N_CORES)

    # ---- I/O ----
    xT = nc.dram_tensor("xT", [D_MODEL, L], F32R, kind="ExternalInput")
    wxcT = nc.dram_tensor("wxcT", [D_MODEL, E], F32R, kind="ExternalInput")
    wzT = nc.dram_tensor("wzT", [D_MODEL, E], F32R, kind="ExternalInput")
    convw = nc.dram_tensor("convw", [E, K_CONV], F32, kind="ExternalInput")
    convb = nc.dram_tensor("convb", [E, 1], F32, kind="ExternalInput")
    wxT = nc.dram_tensor("wxT", [E, DBC], F32R, kind="ExternalInput")
    wdtT = nc.dram_tensor("wdtT", [DT_RANK, E], BF16, kind="ExternalInput")
    bdt = nc.dram_tensor("bdt", [E, 1], F32, kind="ExternalInput")
    acols = nc.dram_tensor("acols", [E, N_ST], F32, kind="ExternalInput")
    dcol = nc.dram_tensor("dcol", [E, 1], F32, kind="ExternalInput")
    woutT = nc.dram_tensor("woutT", [E, D_MODEL], F32R, kind="ExternalInput")
    ident = nc.dram_tensor("ident", [128, 128], BF16, kind="ExternalInput")
    out_pT = nc.dram_tensor("out_pT", [NE, D_MODEL, L], F32, kind="ExternalOutput")

    # internal DRAM: collective buffers + bf16 B/C rows for broadcast
    cc_in = nc.dram_tensor("cc_in", [DBC, L], BF16)
    cc_out = nc.dram_tensor("cc_out", [DBC, L], BF16, addr_space="Shared")
    cc_out_ap = cc_out.ap()

    def ebl(tile3, m):
        return tile3[:, m, :]

    with TileContext(nc) as tc:
        with (
            tc.tile_pool(name="res", bufs=1) as res,       # cross-phase residents
            tc.tile_pool(name="prm", bufs=1) as prm,       # small params
        ):
            # residents
            xc_t = res.tile([128, NE, L], F32R, tag="xc")
            zs_t = res.tile([128, NE, L], F32, tag="zs")
            delta_t = res.tile([128, NE, L], F32, tag="delta")
            wu_t = res.tile([128, NE, L], BF16, tag="wu")

            # params
            convw_t = prm.tile([128, NE, K_CONV], F32, tag="convw")
            convb_t = prm.tile([128, NE, 1], F32, tag="convb")
            bdt_t = prm.tile([128, NE, 1], F32, tag="bdt")
            acols_t = prm.tile([128, NE, N_ST], F32, tag="acols")
            dcol_t = prm.tile([128, NE, 1], F32, tag="dcol")
            wx_t = prm.tile([128, NE, DBC], F32R, tag="wx")
            wdt_t = prm.tile([DT_RANK, E], BF16, tag="wdt")
            ident_t = prm.tile([128, 128], BF16, tag="ident")
            for m in range(NE):
                sl = slice(m * 128, (m + 1) * 128)
                nc.sync.dma_start(out=convw_t[:, m, :], in_=convw[sl, :])
                nc.sync.dma_start(out=convb_t[:, m, :], in_=convb[sl, :])
                nc.sync.dma_start(out=bdt_t[:, m, :], in_=bdt[sl, :])
                nc.sync.dma_start(out=acols_t[:, m, :], in_=acols[sl, :])
                nc.sync.dma_start(out=dcol_t[:, m, :], in_=dcol[sl, :])
                nc.sync.dma_start(out=wx_t[:, m, :], in_=wxT[sl, :])
            nc.sync.dma_start(out=wdt_t[:, :], in_=wdtT[:, :])
            nc.sync.dma_start(out=ident_t[:, :], in_=ident[:, :])

            # =================== PHASE A ===================
            with (
                tc.tile_pool(name="xin", bufs=1) as xin,
                tc.tile_pool(name="wts", bufs=1) as wts,
                tc.tile_pool(name="stg", bufs=2) as stg,
                tc.tile_pool(name="stg1", bufs=1) as stg1,
                tc.tile_pool(name="psA", bufs=1, space="PSUM") as psA,
            ):
                w_in_t = wts.tile([128, 8, 2 * E], F32R, tag="w_in")
                for k in range(8):
                    ksl = slice(k * 128, (k + 1) * 128)
                    nc.sync.dma_start(out=w_in_t[:, k, 0:E], in_=wxcT[ksl, :])
                    nc.sync.dma_start(out=w_in_t[:, k, E:2 * E], in_=wzT[ksl, :])
                x_t = xin.tile([128, 8, L], F32R, tag="x")
                for k in range(8):
                    nc.sync.dma_start(out=x_t[:, k, :], in_=xT[k * 128:(k + 1) * 128, :])

                # ---- in_proj xc half (k-outer: start after first chunks) ----
                psx = [[psA.tile([128, 512], F32, tag=f"pA{m}{t}",
                                 name=f"psx{m}{t}") for t in range(NT)]
                       for m in range(NE)]
                for k in range(8):
                    for m in range(NE):
                        for t in range(NT):
                            nc.tensor.matmul(psx[m][t][:],
                                             w_in_t[:, k, m * 128:(m + 1) * 128],
                                             x_t[:, k, t * 512:(t + 1) * 512],
                                             start=(k == 0), stop=(k == 7))
                for m in range(NE):
                    raw = stg.tile([128, L], F32, tag="xcraw")
                    for t in range(NT):
                        dst = raw[:, t * 512:(t + 1) * 512]
                        if t % 2 == 0:
                            nc.scalar.copy(dst, psx[m][t][:])
                        else:
                            nc.vector.tensor_copy(dst, psx[m][t][:])
                    # causal depthwise conv K=4 + bias + SiLU
                    acc = stg1.tile([128, L], F32, tag="convacc")
                    cw = convw_t[:, m, :]
                    nc.vector.tensor_scalar_mul(acc[:, :], raw[:, :], cw[:, 3:4])
                    for kk in range(1, K_CONV):
                        nc.vector.scalar_tensor_tensor(
                            acc[:, kk:], raw[:, :L - kk], cw[:, 3 - kk:4 - kk],
                            acc[:, kk:], OP.mult, OP.add)
                    nc.scalar.activation(ebl(xc_t, m), acc[:, :], AF.Silu,
                                         bias=convb_t[:, m, :], scale=1.0)

                # ---- x_proj partial -> AllReduce ----
                for t in range(NT):
                    ps = psA.tile([128, 512], F32, tag=f"pA0{t}", name=f"psb{t}")
                    for m in range(NE):
                        nc.tensor.matmul(ps[0:DBC, :], wx_t[:, m, :],
                                         ebl(xc_t, m)[:, t * 512:(t + 1) * 512],
                                         start=(m == 0), stop=(m == NE - 1))
                    dst = stg.tile([DBC, 512], BF16, tag="dbcp")
                    nc.scalar.copy(dst[:, :], ps[0:DBC, :])
                    nc.sync.dma_start(out=cc_in[:, t * 512:(t + 1) * 512], in_=dst[:, :])
                nc.gpsimd.collective_compute(
                    "AllReduce", OP.add,
                    ins=[cc_in[:, :]], outs=[cc_out[:, :]],
                    replica_groups=[list(range(N_CORES))],
                )

                # ---- z half of in_proj + silu (overlaps AllReduce) ----
                for m in range(NE):
                    zraw = stg.tile([128, L], F32, tag="zraw")
                    for t in range(NT):
                        ps = psA.tile([128, 512], F32, tag=f"pA{m}{t}", name=f"psz{m}{t}")
                        for k in range(8):
                            nc.tensor.matmul(ps[:],
                                             w_in_t[:, k, E + m * 128:E + (m + 1) * 128],
                                             x_t[:, k, t * 512:(t + 1) * 512],
                                             start=(k == 0), stop=(k == 7))
                        dst = zraw[:, t * 512:(t + 1) * 512]
                        if t % 2 == 0:
                            nc.scalar.copy(dst, ps[:])
                        else:
                            nc.vector.tensor_copy(dst, ps[:])
                    nc.scalar.activation(ebl(zs_t, m), zraw[:, :], AF.Silu)

            # =================== post-AllReduce ===================
            with (
                tc.tile_pool(name="stg2", bufs=2) as stg2,
                tc.tile_pool(name="psD", bufs=4, space="PSUM") as psD,
            ):
                dbc_r = stg2.tile([DT_RANK, L], BF16, tag="dbc_r")
                nc.sync.dma_start(out=dbc_r[:, :], in_=cc_out[0:DT_RANK, :])

                # delta = softplus(wdt.T @ dbc_r + bdt) via Exp/+1/Ln
                for m in range(NE):
                    dd = ebl(delta_t, m)
                    for t in range(NT):
                        ps = psD.tile([128, 512], F32, tag="pD")
                        nc.tensor.matmul(ps[:], wdt_t[:, m * 128:(m + 1) * 128],
                                         dbc_r[:, t * 512:(t + 1) * 512],
                                         start=True, stop=True)
                        nc.scalar.activation(dd[:, t * 512:(t + 1) * 512], ps[:],
                                             AF.Exp, bias=bdt_t[:, m, :], scale=1.0)
                    nc.vector.tensor_scalar_add(dd, dd, 1.0)
                    nc.scalar.activation(dd, dd, AF.Ln)
                    # w_u = alpha * delta * xc (bf16)
                    nc.vector.tensor_tensor(out=ebl(wu_t, m), in0=dd,
                                            in1=ebl(xc_t, m).bitcast(F32),
                                            op=OP.mult)

            # =================== PHASE B: scan (m outer) ===================
            with (
                tc.tile_pool(name="pb1", bufs=1) as pb1,
                tc.tile_pool(name="rep", bufs=3) as rep,
                tc.tile_pool(name="sc", bufs=3) as sc,
                tc.tile_pool(name="oc", bufs=2) as oc,
                tc.tile_pool(name="psY", bufs=1, space="PSUM") as psY,
                tc.tile_pool(name="psC", bufs=4, space="PSUM") as psC,
            ):
                beta_t = pb1.tile([128, L], BF16, tag="beta")
                nc.vector.memset(beta_t[:, :], BETA)
                wout_t = pb1.tile([128, NE, D_MODEL], F32R, tag="wout")
                for m in range(NE):
                    nc.sync.dma_start(out=wout_t[:, m, :],
                                      in_=woutT[m * 128:(m + 1) * 128, :])

                for m in range(NE):
                    y_ps = psY.tile([128, L], F32, tag="y", name=f"y_ps{m}")
                    for n in range(N_ST):
                        bm_rep = rep.tile([128, L], BF16, tag="bm")
                        cm_rep = rep.tile([128, L], BF16, tag="cm")
                        nc.sync.dma_start(
                            out=bm_rep[:, :],
                            in_=bass.AP(tensor=cc_out_ap.tensor,
                                        offset=(DT_RANK + n) * L,
                                        ap=[[0, 128], [1, L]]))
                        nc.sync.dma_start(
                            out=cm_rep[:, :],
                            in_=bass.AP(tensor=cc_out_ap.tensor,
                                        offset=(DT_RANK + N_ST + n) * L,
                                        ap=[[0, 128], [1, L]]))
                        a_t = sc.tile([128, L], BF16, tag="a")
                        nc.scalar.activation(a_t[:, :], ebl(delta_t, m), AF.Exp,
                                             scale=acols_t[:, m, n:n + 1])
                        u_t = sc.tile([128, L], BF16, tag="u")
                        nc.gpsimd.tensor_tensor(out=u_t[:, :], in0=ebl(wu_t, m),
                                                in1=bm_rep[:, :], op=OP.mult)
                        v_t = sc.tile([128, L], BF16, tag="v")
                        nc.vector.tensor_tensor_scan(v_t[:, :], beta_t[:, :],
                                                     u_t[:, :], 0.0,
                                                     OP.mult, OP.add)
                        h_t = sc.tile([128, L], BF16, tag="h")
                        nc.vector.tensor_tensor_scan(h_t[:, :], a_t[:, :],
                                                     v_t[:, :], 0.0,
                                                     OP.mult, OP.add)
                        yterm = sc.tile([128, L], BF16, tag="yt")
                        nc.vector.tensor_tensor(out=yterm[:, :], in0=h_t[:, :],
                                                in1=cm_rep[:, :], op=OP.mult)
                        for t in range(NT):
                            nc.tensor.matmul(y_ps[:, t * 512:(t + 1) * 512],
                                             ident_t[:, :],
                                             yterm[:, t * 512:(t + 1) * 512],
                                             start=(n == 0), stop=(n == N_ST - 1))

                    # ---- y + D*xc, gate, out_proj partial for this m ----
                    yd = sc.tile([128, L], F32, tag="yd", bufs=2)
                    nc.vector.scalar_tensor_tensor(
                        yd[:, :], ebl(xc_t, m).bitcast(F32), dcol_t[:, m, :],
                        y_ps[:, :], OP.mult, OP.add)
                    g_t = sc.tile([128, L], F32R, tag="g", bufs=2)
                    nc.vector.tensor_tensor(out=g_t[:, :], in0=yd[:, :],
                                            in1=ebl(zs_t, m), op=OP.mult)
                    for mo in range(8):
                        for t in range(NT):
                            ps = psC.tile([128, 512], F32, tag="pC")
                            nc.tensor.matmul(
                                ps[:],
                                wout_t[:, m, mo * 128:(mo + 1) * 128],
                                g_t[:, t * 512:(t + 1) * 512],
                                start=True, stop=True)
                            ot = oc.tile([128, 512], F32, tag="ot")
                            if (mo + t) % 2 == 0:
                                nc.scalar.copy(ot[:, :], ps[:])
                            else:
                                nc.vector.tensor_copy(ot[:, :], ps[:])
                            nc.sync.dma_start(
                                out=out_pT[m, mo * 128:(mo + 1) * 128,
                                           t * 512:(t + 1) * 512],
                                in_=ot[:, :])

    _split_ctrl_waits(nc)
    return nc


def _get_program():
    if "nc" not in _CACHE:
        _CACHE["nc"] = _build_program()
    return _CACHE["nc"]


def _in_maps(x, W_in, conv_w, conv_b, W_x, W_dt, b_dt, A_log, D, W_out):
    x = np.asarray(x, np.float32)
    xT = np.ascontiguousarray(x[0].T)                       # (1024, 2048)
    A = -np.exp(np.asarray(A_log, np.float32))              # (ED, N)
    ident = np.eye(128, dtype=ml_dtypes.bfloat16)
    W_in = np.asarray(W_in, np.float32)

    in_maps = []
    for j in range(N_CORES):
        sl = slice(j * E, (j + 1) * E)
        in_maps.append({
            "xT": xT,
            "wxcT": np.ascontiguousarray(W_in[sl, :].T),
            "wzT": np.ascontiguousarray(W_in[ED + j * E:ED + (j + 1) * E, :].T),
            "convw": np.ascontiguousarray(np.asarray(conv_w, np.float32)[sl]),
            "convb": np.ascontiguousarray(np.asarray(conv_b, np.float32)[sl])[:, None],
            "wxT": np.ascontiguousarray(np.asarray(W_x, np.float32)[:, sl].T),
            "wdtT": np.ascontiguousarray(np.asarray(W_dt, np.float32)[sl, :].T).astype(ml_dtypes.bfloat16),
            "bdt": np.ascontiguousarray(np.asarray(b_dt, np.float32)[sl])[:, None],
            "acols": np.ascontiguousarray(A[sl, :]),
            "dcol": np.ascontiguousarray(np.asarray(D, np.float32)[sl])[:, None],
            "woutT": np.ascontiguousarray(np.asarray(W_out, np.float32)[:, sl].T),
            "ident": ident,
        })
    return in_maps


def kernel(x, W_in, conv_w, conv_b, W_x, W_dt, b_dt, A_log, D, W_out):
    from concourse.bass_utils import run_bass_kernel_spmd

    nc = _get_program()
    in_maps = _in_maps(x, W_in, conv_w, conv_b, W_x, W_dt, b_dt, A_log, D, W_out)
    res = run_bass_kernel_spmd(nc, in_maps, list(range(N_CORES)))
    out_T = np.zeros((D_MODEL, L), np.float64)
    for j in range(N_CORES):
        out_T += res.results[j]["out_pT"].sum(axis=0)
    return out_T.T[None, :, :].astype(np.float32)


# revision 12
# speedup vs baseline: 1.3111x; 1.3111x over previous
"""Trainium2 Bass kernel for MambaMomentum (B=1, L=2048, D=1024, ED=2048, N=16).

Strategy: tensor-parallel over d_inner (ED) across 8 NeuronCores.
Each core owns 256 ED channels: in_proj columns, depthwise conv, the
(ED x N) selective scan with momentum, and out_proj rows. The only
cross-core dependency is dBC = xc @ W_x.T (a full-ED contraction),
handled with one small on-device AllReduce (2048 x 96 fp32). out_proj
partial products are summed on the host (partial-sum unshard).

Scan mapping: e-channels on partitions, time on the free dimension;
DVE TensorTensorScan computes h_t = a_t * h_{t-1} + v_t directly.
The momentum recurrence v_t = beta*v_{t-1} + u_t is the same scan with
a constant decay.
"""

import sys

if "/opt/trn_rl_repo" not in sys.path:
    sys.path.insert(0, "/opt/trn_rl_repo")

import numpy as np
import ml_dtypes

import concourse.bass as bass
import concourse.mybir as mybir
from concourse.tile import TileContext

N_CORES = 8
D_MODEL = 1024
ED = 2048
N_ST = 16          # d_state
DT_RANK = 64
K_CONV = 4
BETA = 0.6
ALPHA = 1.0
L = 2048
E = ED // N_CORES  # 256 channels per core
NE = E // 128      # 2 e-tiles per core
NT = L // 512      # 4 t-chunks
DBC = DT_RANK + 2 * N_ST  # 96
BF16 = mybir.dt.bfloat16
F32 = mybir.dt.float32
F32R = mybir.dt.float32r
AF = mybir.ActivationFunctionType
OP = mybir.AluOpType

_CACHE = {}


def _split_ctrl_waits(nc, max_waits=1):
    """walrus CoreV3 codegen rejects instructions with >1 sem-wait on several
    encodings; move excess waits onto single-wait NoOps just before."""
    for fn in nc.m.functions:
        for bb in fn.blocks:
            new_insts = []
            for inst in bb.instructions:
                si = inst.sync_info
                if si is not None and si.on_wait and len(si.on_wait) > max_waits:
                    waits = list(si.on_wait)
                    si.on_wait = waits[:max_waits]
                    extra = waits[max_waits:]
                    for i in range(0, len(extra), max_waits):
                        new_insts.append(mybir.InstNoOp(
                            name=f"{inst.name}_ws{i}",
                            engine=inst.engine,
                            ins=[], outs=[],
                            sync_info=mybir.SyncInfo(
                                on_wait=extra[i:i + max_waits], on_update=[]),
                        ))
                new_insts.append(inst)
            bb.instructions[:] = new_insts


def _build_program():
    nc = bass.Bass("TRN2", target_bir_lowering=False, debug=False,
                   num_devices=N_CORES)

    # ---- I/O ----
    xT = nc.dram_tensor("xT", [D_MODEL, L], F32R, kind="ExternalInput")
    wxcT = nc.dram_tensor("wxcT", [D_MODEL, E], F32R, kind="ExternalInput")
    wzT = nc.dram_tensor("wzT", [D_MODEL, E], F32R, kind="ExternalInput")
    convw = nc.dram_tensor("convw", [E, K_CONV], F32, kind="ExternalInput")
    convb = nc.dram_tensor("convb", [E, 1], F32, kind="ExternalInput")
    wxT = nc.dram_tensor("wxT", [E, DBC], F32R, kind="ExternalInput")
    wdtT = nc.dram_tensor("wdtT", [DT_RANK, E], BF16, kind="ExternalInput")
    bdt = nc.dram_tensor("bdt", [E, 1], F32, kind="ExternalInput")
    acols = nc.dram_tensor("acols", [E, N_ST], F32, kind="ExternalInput")
    dcol = nc.dram_tensor("dcol", [E, 1], F32, kind="ExternalInput")
    woutT = nc.dram_tensor("woutT", [E, D_MODEL], F32R, kind="ExternalInput")
    ident = nc.dram_tensor("ident", [128, 128], BF16, kind="ExternalInput")
    out_pT = nc.dram_tensor("out_pT", [D_MODEL, L], F32, kind="ExternalOutput")

    # internal DRAM: collective buffers + bf16 B/C rows for broadcast
    cc_in = nc.dram_tensor("cc_in", [DBC, L], F32)
    cc_out = nc.dram_tensor("cc_out", [DBC, L], F32, addr_space="Shared")
    bcrows = nc.dram_tensor("bcrows", [2 * N_ST, L], BF16)
    bcrows_ap = bcrows.ap()

    def ebl(tile3, m):
        return tile3[:, m, :]

    with TileContext(nc) as tc:
        with (
            tc.tile_pool(name="res", bufs=1) as res,       # cross-phase residents
            tc.tile_pool(name="prm", bufs=1) as prm,       # small params
        ):
            # residents
            xc_t = res.tile([128, NE, L], F32R, tag="xc")
            zs_t = res.tile([128, NE, L], F32, tag="zs")
            delta_t = res.tile([128, NE, L], F32, tag="delta")
            wu_t = res.tile([128, NE, L], BF16, tag="wu")

            # params
            convw_t = prm.tile([128, NE, K_CONV], F32, tag="convw")
            convb_t = prm.tile([128, NE, 1], F32, tag="convb")
            bdt_t = prm.tile([128, NE, 1], F32, tag="bdt")
            acols_t = prm.tile([128, NE, N_ST], F32, tag="acols")
            dcol_t = prm.tile([128, NE, 1], F32, tag="dcol")
            wx_t = prm.tile([128, NE, DBC], F32R, tag="wx")
            wdt_t = prm.tile([DT_RANK, E], BF16, tag="wdt")
            ident_t = prm.tile([128, 128], BF16, tag="ident")
            for m in range(NE):
                sl = slice(m * 128, (m + 1) * 128)
                nc.sync.dma_start(out=convw_t[:, m, :], in_=convw[sl, :])
                nc.sync.dma_start(out=convb_t[:, m, :], in_=convb[sl, :])
                nc.sync.dma_start(out=bdt_t[:, m, :], in_=bdt[sl, :])
                nc.sync.dma_start(out=acols_t[:, m, :], in_=acols[sl, :])
                nc.sync.dma_start(out=dcol_t[:, m, :], in_=dcol[sl, :])
                nc.sync.dma_start(out=wx_t[:, m, :], in_=wxT[sl, :])
            nc.sync.dma_start(out=wdt_t[:, :], in_=wdtT[:, :])
            nc.sync.dma_start(out=ident_t[:, :], in_=ident[:, :])

            # =================== PHASE A ===================
            with (
                tc.tile_pool(name="xin", bufs=1) as xin,
                tc.tile_pool(name="wts", bufs=1) as wts,
                tc.tile_pool(name="stg", bufs=2) as stg,
                tc.tile_pool(name="stg1", bufs=1) as stg1,
                tc.tile_pool(name="psA", bufs=1, space="PSUM") as psA,
            ):
                w_in_t = wts.tile([128, 8, 2 * E], F32R, tag="w_in")
                for k in range(8):
                    ksl = slice(k * 128, (k + 1) * 128)
                    nc.sync.dma_start(out=w_in_t[:, k, 0:E], in_=wxcT[ksl, :])
                    nc.sync.dma_start(out=w_in_t[:, k, E:2 * E], in_=wzT[ksl, :])
                x_t = xin.tile([128, 8, L], F32R, tag="x")
                for k in range(8):
                    nc.sync.dma_start(out=x_t[:, k, :], in_=xT[k * 128:(k + 1) * 128, :])

                # ---- in_proj xc half (k-outer: start after first chunks) ----
                psx = [[psA.tile([128, 512], F32, tag=f"pA{m}{t}",
                                 name=f"psx{m}{t}") for t in range(NT)]
                       for m in range(NE)]
                for k in range(8):
                    for m in range(NE):
                        for t in range(NT):
                            nc.tensor.matmul(psx[m][t][:],
                                             w_in_t[:, k, m * 128:(m + 1) * 128],
                                             x_t[:, k, t * 512:(t + 1) * 512],
                                             start=(k == 0), stop=(k == 7))
                for m in range(NE):
                    raw = stg.tile([128, L], F32, tag="xcraw")
                    for t in range(NT):
                        dst = raw[:, t * 512:(t + 1) * 512]
                        if t % 2 == 0:
                            nc.scalar.copy(dst, psx[m][t][:])
                        else:
                            nc.vector.tensor_copy(dst, psx[m][t][:])
                    # causal depthwise conv K=4 + bias + SiLU
                    acc = stg1.tile([128, L], F32, tag="convacc")
                    cw = convw_t[:, m, :]
                    nc.vector.tensor_scalar_mul(acc[:, :], raw[:, :], cw[:, 3:4])
                    for kk in range(1, K_CONV):
                        nc.vector.scalar_tensor_tensor(
                            acc[:, kk:], raw[:, :L - kk], cw[:, 3 - kk:4 - kk],
                            acc[:, kk:], OP.mult, OP.add)
                    nc.scalar.activation(ebl(xc_t, m), acc[:, :], AF.Silu,
                                         bias=convb_t[:, m, :], scale=1.0)

                # ---- x_proj partial -> AllReduce ----
                for t in range(NT):
                    ps = psA.tile([128, 512], F32, tag=f"pA0{t}", name=f"psb{t}")
                    for m in range(NE):
                        nc.tensor.matmul(ps[0:DBC, :], wx_t[:, m, :],
                                         ebl(xc_t, m)[:, t * 512:(t + 1) * 512],
                                         start=(m == 0), stop=(m == NE - 1))
                    dst = stg.tile([DBC, 512], F32, tag="dbcp")
                    nc.scalar.copy(dst[:, :], ps[0:DBC, :])
                    nc.sync.dma_start(out=cc_in[:, t * 512:(t + 1) * 512], in_=dst[:, :])
                nc.gpsimd.collective_compute(
                    "AllReduce", OP.add,
                    ins=[cc_in[:, :]], outs=[cc_out[:, :]],
                    replica_groups=[list(range(N_CORES))],
                )

                # ---- z half of in_proj + silu (overlaps AllReduce) ----
                for m in range(NE):
                    zraw = stg.tile([128, L], F32, tag="zraw")
                    for t in range(NT):
                        ps = psA.tile([128, 512], F32, tag=f"pA{m}{t}", name=f"psz{m}{t}")
                        for k in range(8):
                            nc.tensor.matmul(ps[:],
                                             w_in_t[:, k, E + m * 128:E + (m + 1) * 128],
                                             x_t[:, k, t * 512:(t + 1) * 512],
                                             start=(k == 0), stop=(k == 7))
                        dst = zraw[:, t * 512:(t + 1) * 512]
                        if t % 2 == 0:
                            nc.scalar.copy(dst, ps[:])
                        else:
                            nc.vector.tensor_copy(dst, ps[:])
                    nc.scalar.activation(ebl(zs_t, m), zraw[:, :], AF.Silu)

            # =================== post-AllReduce ===================
            with (
                tc.tile_pool(name="stg2", bufs=2) as stg2,
                tc.tile_pool(name="psD", bufs=4, space="PSUM") as psD,
            ):
                dbc_t = stg2.tile([DBC, L], F32, tag="dbc")
                nc.sync.dma_start(out=dbc_t[:, :], in_=cc_out[:, :])
                dbc_r = stg2.tile([DT_RANK, L], BF16, tag="dbc_r")
                nc.vector.tensor_copy(dbc_r[:, :], dbc_t[0:DT_RANK, :])
                bc_bf = stg2.tile([2 * N_ST, L], BF16, tag="bc_bf")
                nc.vector.tensor_copy(bc_bf[:, :], dbc_t[DT_RANK:DBC, :])
                nc.sync.dma_start(out=bcrows[:, :], in_=bc_bf[:, :])

                # delta = softplus(wdt.T @ dbc_r + bdt) via Exp/+1/Ln
                for m in range(NE):
                    dd = ebl(delta_t, m)
                    for t in range(NT):
                        ps = psD.tile([128, 512], F32, tag="pD")
                        nc.tensor.matmul(ps[:], wdt_t[:, m * 128:(m + 1) * 128],
                                         dbc_r[:, t * 512:(t + 1) * 512],
                                         start=True, stop=True)
                        nc.scalar.activation(dd[:, t * 512:(t + 1) * 512], ps[:],
                                             AF.Exp, bias=bdt_t[:, m, :], scale=1.0)
                    nc.vector.tensor_scalar_add(dd, dd, 1.0)
                    nc.scalar.activation(dd, dd, AF.Ln)
                    # w_u = alpha * delta * xc (bf16)
                    nc.vector.tensor_tensor(out=ebl(wu_t, m), in0=dd,
                                            in1=ebl(xc_t, m).bitcast(F32),
                                            op=OP.mult)

            # =================== PHASE B: scan ===================
            with (
                tc.tile_pool(name="pb1", bufs=1) as pb1,
                tc.tile_pool(name="rep", bufs=3) as rep,
                tc.tile_pool(name="sc", bufs=3) as sc,
                tc.tile_pool(name="psY", bufs=1, space="PSUM") as psY,
            ):
                beta_t = pb1.tile([128, L], BF16, tag="beta")
                nc.vector.memset(beta_t[:, :], BETA)
                wout_t = res.tile([128, NE, D_MODEL], F32R, tag="wout")
                for m in range(NE):
                    nc.sync.dma_start(out=wout_t[:, m, :],
                                      in_=woutT[m * 128:(m + 1) * 128, :])
                y_ps = [psY.tile([128, L], F32, tag=f"y{m}", name=f"y_ps{m}")
                        for m in range(NE)]

                for n in range(N_ST):
                    bm_rep = rep.tile([128, L], BF16, tag="bm")
                    cm_rep = rep.tile([128, L], BF16, tag="cm")
                    nc.sync.dma_start(
                        out=bm_rep[:, :],
                        in_=bass.AP(tensor=bcrows_ap.tensor, offset=n * L,
                                    ap=[[0, 128], [1, L]]))
                    nc.sync.dma_start(
                        out=cm_rep[:, :],
                        in_=bass.AP(tensor=bcrows_ap.tensor, offset=(N_ST + n) * L,
                                    ap=[[0, 128], [1, L]]))
                    for m in range(NE):
                        a_t = sc.tile([128, L], BF16, tag="a")
                        nc.scalar.activation(a_t[:, :], ebl(delta_t, m), AF.Exp,
                                             scale=acols_t[:, m, n:n + 1])
                        u_t = sc.tile([128, L], BF16, tag="u")
                        nc.gpsimd.tensor_tensor(out=u_t[:, :], in0=ebl(wu_t, m),
                                                in1=bm_rep[:, :], op=OP.mult)
                        v_t = sc.tile([128, L], BF16, tag="v")
                        nc.vector.tensor_tensor_scan(v_t[:, :], beta_t[:, :],
                                                     u_t[:, :], 0.0,
                                                     OP.mult, OP.add)
                        h_t = sc.tile([128, L], BF16, tag="h")
                        nc.vector.tensor_tensor_scan(h_t[:, :], a_t[:, :],
                                                     v_t[:, :], 0.0,
                                                     OP.mult, OP.add)
                        yterm = sc.tile([128, L], BF16, tag="yt")
                        nc.vector.tensor_tensor(out=yterm[:, :], in0=h_t[:, :],
                                                in1=cm_rep[:, :], op=OP.mult)
                        for t in range(NT):
                            nc.tensor.matmul(y_ps[m][:, t * 512:(t + 1) * 512],
                                             ident_t[:, :],
                                             yterm[:, t * 512:(t + 1) * 512],
                                             start=(n == 0), stop=(n == N_ST - 1))

                # ---- y + D*xc, gate (inside psY scope) ----
                g_t = res.tile([128, NE, L], F32R, tag="g")
                for m in range(NE):
                    yd = sc.tile([128, L], F32, tag="yd", bufs=2)
                    nc.vector.scalar_tensor_tensor(
                        yd[:, :], ebl(xc_t, m).bitcast(F32), dcol_t[:, m, :],
                        y_ps[m][:, :], OP.mult, OP.add)
                    nc.vector.tensor_tensor(out=ebl(g_t, m), in0=yd[:, :],
                                            in1=ebl(zs_t, m), op=OP.mult)

            # =================== PHASE C: out_proj ===================
            with (
                tc.tile_pool(name="oc", bufs=4) as oc,
                tc.tile_pool(name="psC", bufs=4, space="PSUM") as psC,
            ):
                for mo in range(8):
                    for t in range(NT):
                        ps = psC.tile([128, 512], F32, tag="pC")
                        for m in range(NE):
                            nc.tensor.matmul(
                                ps[:],
                                wout_t[:, m, mo * 128:(mo + 1) * 128],
                                ebl(g_t, m)[:, t * 512:(t + 1) * 512],
                                start=(m == 0), stop=(m == NE - 1))
                        ot = oc.tile([128, 512], F32, tag="ot")
                        if (mo + t) % 2 == 0:
                            nc.scalar.copy(ot[:, :], ps[:])
                        else:
                            nc.vector.tensor_copy(ot[:, :], ps[:])
                        nc.sync.dma_start(
                            out=out_pT[mo * 128:(mo + 1) * 128,
                                       t * 512:(t + 1) * 512],
                            in_=ot[:, :])

    _split_ctrl_waits(nc)
    return nc


def _get_program():
    if "nc" not in _CACHE:
        _CACHE["nc"] = _build_program()
    return _CACHE["nc"]


def _in_maps(x, W_in, conv_w, conv_b, W_x, W_dt, b_dt, A_log, D, W_out):
    x = np.asarray(x, np.float32)
    xT = np.ascontiguousarray(x[0].T)                       # (1024, 2048)
    A = -np.exp(np.asarray(A_log, np.float32))              # (ED, N)
    ident = np.eye(128, dtype=ml_dtypes.bfloat16)
    W_in = np.asarray(W_in, np.float32)

    in_maps = []
    for j in range(N_CORES):
        sl = slice(j * E, (j + 1) * E)
        in_maps.append({
            "xT": xT,
            "wxcT": np.ascontiguousarray(W_in[sl, :].T),
            "wzT": np.ascontiguousarray(W_in[ED + j * E:ED + (j + 1) * E, :].T),
            "convw": np.ascontiguousarray(np.asarray(conv_w, np.float32)[sl]),
            "convb": np.ascontiguousarray(np.asarray(conv_b, np.float32)[sl])[:, None],
            "wxT": np.ascontiguousarray(np.asarray(W_x, np.float32)[:, sl].T),
            "wdtT": np.ascontiguousarray(np.asarray(W_dt, np.float32)[sl, :].T).astype(ml_dtypes.bfloat16),
            "bdt": np.ascontiguousarray(np.asarray(b_dt, np.float32)[sl])[:, None],
            "acols": np.ascontiguousarray(A[sl, :]),
            "dcol": np.ascontiguousarray(np.asarray(D, np.float32)[sl])[:, None],
            "woutT": np.ascontiguousarray(np.asarray(W_out, np.float32)[:, sl].T),
            "ident": ident,
        })
    return in_maps


def kernel(x, W_in, conv_w, conv_b, W_x, W_dt, b_dt, A_log, D, W_out):
    from concourse.bass_utils import run_bass_kernel_spmd

    nc = _get_program()
    in_maps = _in_maps(x, W_in, conv_w, conv_b, W_x, W_dt, b_dt, A_log, D, W_out)
    res = run_bass_kernel_spmd(nc, in_maps, list(range(N_CORES)))
    out_T = np.zeros((D_MODEL, L), np.float64)
    for j in range(N_CORES):
        out_T += res.results[j]["out_pT"]
    return out_T.T[None, :, :].astype(np.float32)


# revision 13
# speedup vs baseline: 1.5287x; 1.1660x over previous
"""Trainium2 Bass kernel for MambaMomentum (B=1, L=2048, D=1024, ED=2048, N=16).

Strategy: tensor-parallel over d_inner (ED) across 8 NeuronCores.
Each core owns 256 ED channels: in_proj columns, depthwise conv, the
(ED x N) selective scan with momentum, and out_proj rows. The only
cross-core dependency is dBC = xc @ W_x.T (a full-ED contraction),
handled with one small on-device AllReduce (2048 x 96 fp32). out_proj
partial products are summed on the host (partial-sum unshard).

Scan mapping: e-channels on partitions, time on the free dimension;
DVE TensorTensorScan computes h_t = a_t * h_{t-1} + v_t directly.
The momentum recurrence v_t = beta*v_{t-1} + u_t is the same scan with
a constant decay.
"""

import sys

if "/opt/trn_rl_repo" not in sys.path:
    sys.path.insert(0, "/opt/trn_rl_repo")

import numpy as np
import ml_dtypes

import concourse.bass as bass
import concourse.mybir as mybir
from concourse.tile import TileContext

N_CORES = 8
D_MODEL = 1024
ED = 2048
N_ST = 16          # d_state
DT_RANK = 64
K_CONV = 4
BETA = 0.6
ALPHA = 1.0
L = 2048
E = ED // N_CORES  # 256 channels per core
NE = E // 128      # 2 e-tiles per core
NT = L // 512      # 4 t-chunks
DBC = DT_RANK + 2 * N_ST  # 96
BF16 = mybir.dt.bfloat16
F32 = mybir.dt.float32
F32R = mybir.dt.float32r
AF = mybir.ActivationFunctionType
OP = mybir.AluOpType

_CACHE = {}


def _split_ctrl_waits(nc, max_waits=1):
    """walrus CoreV3 codegen rejects instructions with >1 sem-wait on several
    encodings; move excess waits onto single-wait NoOps just before."""
    for fn in nc.m.functions:
        for bb in fn.blocks:
            new_insts = []
            for inst in bb.instructions:
                si = inst.sync_info
                if si is not None and si.on_wait and len(si.on_wait) > max_waits:
                    waits = list(si.on_wait)
                    si.on_wait = waits[:max_waits]
                    extra = waits[max_waits:]
                    for i in range(0, len(extra), max_waits):
                        new_insts.append(mybir.InstNoOp(
                            name=f"{inst.name}_ws{i}",
                            engine=inst.engine,
                            ins=[], outs=[],
                            sync_info=mybir.SyncInfo(
                                on_wait=extra[i:i + max_waits], on_update=[]),
                        ))
                new_insts.append(inst)
            bb.instructions[:] = new_insts


def _build_program():
    nc = bass.Bass("TRN2", target_bir_lowering=False, debug=False,
                   num_devices=N_CORES)

    # ---- I/O ----
    xT = nc.dram_tensor("xT", [D_MODEL, L], F32R, kind="ExternalInput")
    wxcT = nc.dram_tensor("wxcT", [D_MODEL, E], F32R, kind="ExternalInput")
    wzT = nc.dram_tensor("wzT", [D_MODEL, E], F32R, kind="ExternalInput")
    convw = nc.dram_tensor("convw", [E, K_CONV], F32, kind="ExternalInput")
    convb = nc.dram_tensor("convb", [E, 1], F32, kind="ExternalInput")
    wxT = nc.dram_tensor("wxT", [E, DBC], F32R, kind="ExternalInput")
    wdtT = nc.dram_tensor("wdtT", [DT_RANK, E], BF16, kind="ExternalInput")
    bdt = nc.dram_tensor("bdt", [E, 1], F32, kind="ExternalInput")
    acols = nc.dram_tensor("acols", [E, N_ST], F32, kind="ExternalInput")
    dcol = nc.dram_tensor("dcol", [E, 1], F32, kind="ExternalInput")
    woutT = nc.dram_tensor("woutT", [E, D_MODEL], F32R, kind="ExternalInput")
    ident = nc.dram_tensor("ident", [128, 128], BF16, kind="ExternalInput")
    out_pT = nc.dram_tensor("out_pT", [D_MODEL, L], F32, kind="ExternalOutput")

    # internal DRAM: collective buffers + bf16 B/C rows for broadcast
    cc_in = nc.dram_tensor("cc_in", [DBC, L], F32)
    cc_out = nc.dram_tensor("cc_out", [DBC, L], F32, addr_space="Shared")
    bcrows = nc.dram_tensor("bcrows", [2 * N_ST, L], BF16)
    bcrows_ap = bcrows.ap()

    def ebl(tile3, m):
        return tile3[:, m, :]

    with TileContext(nc) as tc:
        with (
            tc.tile_pool(name="res", bufs=1) as res,       # cross-phase residents
            tc.tile_pool(name="prm", bufs=1) as prm,       # small params
        ):
            # residents
            xc_t = res.tile([128, NE, L], F32R, tag="xc")
            zs_t = res.tile([128, NE, L], F32, tag="zs")
            delta_t = res.tile([128, NE, L], F32, tag="delta")
            wu_t = res.tile([128, NE, L], BF16, tag="wu")

            # params
            convw_t = prm.tile([128, NE, K_CONV], F32, tag="convw")
            convb_t = prm.tile([128, NE, 1], F32, tag="convb")
            bdt_t = prm.tile([128, NE, 1], F32, tag="bdt")
            acols_t = prm.tile([128, NE, N_ST], F32, tag="acols")
            dcol_t = prm.tile([128, NE, 1], F32, tag="dcol")
            wx_t = prm.tile([128, NE, DBC], F32R, tag="wx")
            wdt_t = prm.tile([DT_RANK, E], BF16, tag="wdt")
            ident_t = prm.tile([128, 128], BF16, tag="ident")
            for m in range(NE):
                sl = slice(m * 128, (m + 1) * 128)
                nc.gpsimd.dma_start(out=convw_t[:, m, :], in_=convw[sl, :])
                nc.gpsimd.dma_start(out=convb_t[:, m, :], in_=convb[sl, :])
                nc.gpsimd.dma_start(out=bdt_t[:, m, :], in_=bdt[sl, :])
                nc.gpsimd.dma_start(out=acols_t[:, m, :], in_=acols[sl, :])
                nc.gpsimd.dma_start(out=dcol_t[:, m, :], in_=dcol[sl, :])
                nc.gpsimd.dma_start(out=wx_t[:, m, :], in_=wxT[sl, :])
            nc.gpsimd.dma_start(out=wdt_t[:, :], in_=wdtT[:, :])
            nc.gpsimd.dma_start(out=ident_t[:, :], in_=ident[:, :])

            # =================== PHASE A ===================
            with (
                tc.tile_pool(name="xin", bufs=1) as xin,
                tc.tile_pool(name="wts", bufs=1) as wts,
                tc.tile_pool(name="stg", bufs=2) as stg,
                tc.tile_pool(name="stg1", bufs=1) as stg1,
                tc.tile_pool(name="psA", bufs=1, space="PSUM") as psA,
            ):
                w_in_t = wts.tile([128, 8, 2 * E], F32R, tag="w_in")
                x_t = xin.tile([128, 8, L], F32R, tag="x")
                for k in range(8):
                    ksl = slice(k * 128, (k + 1) * 128)
                    nc.sync.dma_start(out=w_in_t[:, k, 0:E], in_=wxcT[ksl, :])
                    nc.sync.dma_start(out=x_t[:, k, :], in_=xT[ksl, :])
                    nc.sync.dma_start(out=w_in_t[:, k, E:2 * E], in_=wzT[ksl, :])

                # ---- in_proj xc half (k-outer: start after first chunks) ----
                psx = [[psA.tile([128, 512], F32, tag=f"pA{m}{t}",
                                 name=f"psx{m}{t}") for t in range(NT)]
                       for m in range(NE)]
                for k in range(8):
                    for m in range(NE):
                        for t in range(NT):
                            nc.tensor.matmul(psx[m][t][:],
                                             w_in_t[:, k, m * 128:(m + 1) * 128],
                                             x_t[:, k, t * 512:(t + 1) * 512],
                                             start=(k == 0), stop=(k == 7))
                for m in range(NE):
                    raw = stg.tile([128, L], F32, tag="xcraw")
                    for t in range(NT):
                        nc.scalar.copy(raw[:, t * 512:(t + 1) * 512], psx[m][t][:])
                    # causal depthwise conv K=4 + bias + SiLU
                    acc = stg1.tile([128, L], F32, tag="convacc")
                    cw = convw_t[:, m, :]
                    nc.vector.tensor_scalar_mul(acc[:, :], raw[:, :], cw[:, 3:4])
                    for kk in range(1, K_CONV):
                        nc.vector.scalar_tensor_tensor(
                            acc[:, kk:], raw[:, :L - kk], cw[:, 3 - kk:4 - kk],
                            acc[:, kk:], OP.mult, OP.add)
                    nc.scalar.activation(ebl(xc_t, m), acc[:, :], AF.Silu,
                                         bias=convb_t[:, m, :], scale=1.0)

                # ---- x_proj partial -> AllReduce ----
                for t in range(NT):
                    ps = psA.tile([128, 512], F32, tag=f"pA0{t}", name=f"psb{t}")
                    for m in range(NE):
                        nc.tensor.matmul(ps[0:DBC, :], wx_t[:, m, :],
                                         ebl(xc_t, m)[:, t * 512:(t + 1) * 512],
                                         start=(m == 0), stop=(m == NE - 1))
                    dst = stg.tile([DBC, 512], F32, tag="dbcp")
                    nc.scalar.copy(dst[:, :], ps[0:DBC, :])
                    nc.sync.dma_start(out=cc_in[:, t * 512:(t + 1) * 512], in_=dst[:, :])
                nc.gpsimd.collective_compute(
                    "AllReduce", OP.add,
                    ins=[cc_in[:, :]], outs=[cc_out[:, :]],
                    replica_groups=[list(range(N_CORES))],
                )

                # ---- z half of in_proj + silu (overlaps AllReduce) ----
                for m in range(NE):
                    zraw = stg.tile([128, L], F32, tag="zraw")
                    for t in range(NT):
                        ps = psA.tile([128, 512], F32, tag=f"pA{m}{t}", name=f"psz{m}{t}")
                        for k in range(8):
                            nc.tensor.matmul(ps[:],
                                             w_in_t[:, k, E + m * 128:E + (m + 1) * 128],
                                             x_t[:, k, t * 512:(t + 1) * 512],
                                             start=(k == 0), stop=(k == 7))
                        nc.scalar.copy(zraw[:, t * 512:(t + 1) * 512], ps[:])
                    nc.scalar.activation(ebl(zs_t, m), zraw[:, :], AF.Silu)

            # =================== post-AllReduce ===================
            with (
                tc.tile_pool(name="stg2", bufs=2) as stg2,
                tc.tile_pool(name="psD", bufs=4, space="PSUM") as psD,
            ):
                dbc_t = stg2.tile([DBC, L], F32, tag="dbc")
                nc.sync.dma_start(out=dbc_t[:, :], in_=cc_out[:, :])
                dbc_r = stg2.tile([DT_RANK, L], BF16, tag="dbc_r")
                nc.vector.tensor_copy(dbc_r[:, :], dbc_t[0:DT_RANK, :])
                bc_bf = stg2.tile([2 * N_ST, L], BF16, tag="bc_bf")
                nc.vector.tensor_copy(bc_bf[:, :], dbc_t[DT_RANK:DBC, :])
                nc.sync.dma_start(out=bcrows[:, :], in_=bc_bf[:, :])

                # delta = softplus(wdt.T @ dbc_r + bdt) via Exp/+1/Ln
                for m in range(NE):
                    dd = ebl(delta_t, m)
                    for t in range(NT):
                        ps = psD.tile([128, 512], F32, tag="pD")
                        nc.tensor.matmul(ps[:], wdt_t[:, m * 128:(m + 1) * 128],
                                         dbc_r[:, t * 512:(t + 1) * 512],
                                         start=True, stop=True)
                        nc.scalar.activation(dd[:, t * 512:(t + 1) * 512], ps[:],
                                             AF.Exp, bias=bdt_t[:, m, :], scale=1.0)
                    nc.vector.tensor_scalar_add(dd, dd, 1.0)
                    nc.scalar.activation(dd, dd, AF.Ln)
                    # w_u = alpha * delta * xc (bf16)
                    nc.vector.tensor_tensor(out=ebl(wu_t, m), in0=dd,
                                            in1=ebl(xc_t, m).bitcast(F32),
                                            op=OP.mult)

            # =================== PHASE B: scan ===================
            with (
                tc.tile_pool(name="pb1", bufs=1) as pb1,
                tc.tile_pool(name="rep", bufs=3) as rep,
                tc.tile_pool(name="sc", bufs=3) as sc,
                tc.tile_pool(name="psY", bufs=1, space="PSUM") as psY,
            ):
                beta_t = pb1.tile([128, L], BF16, tag="beta")
                nc.vector.memset(beta_t[:, :], BETA)
                wout_t = res.tile([128, NE, D_MODEL], F32R, tag="wout")
                for m in range(NE):
                    nc.sync.dma_start(out=wout_t[:, m, :],
                                      in_=woutT[m * 128:(m + 1) * 128, :])
                y_ps = [psY.tile([128, L], F32, tag=f"y{m}", name=f"y_ps{m}")
                        for m in range(NE)]

                for n in range(N_ST):
                    bm_rep = rep.tile([128, L], BF16, tag="bm")
                    cm_rep = rep.tile([128, L], BF16, tag="cm")
                    nc.sync.dma_start(
                        out=bm_rep[:, :],
                        in_=bass.AP(tensor=bcrows_ap.tensor, offset=n * L,
                                    ap=[[0, 128], [1, L]]))
                    nc.sync.dma_start(
                        out=cm_rep[:, :],
                        in_=bass.AP(tensor=bcrows_ap.tensor, offset=(N_ST + n) * L,
                                    ap=[[0, 128], [1, L]]))
                    for m in range(NE):
                        a_t = sc.tile([128, L], BF16, tag="a")
                        nc.scalar.activation(a_t[:, :], ebl(delta_t, m), AF.Exp,
                                             scale=acols_t[:, m, n:n + 1])
                        u_t = sc.tile([128, L], BF16, tag="u")
                        nc.vector.tensor_tensor(out=u_t[:, :], in0=ebl(wu_t, m),
                                                in1=bm_rep[:, :], op=OP.mult)
                        v_t = sc.tile([128, L], BF16, tag="v")
                        nc.vector.tensor_tensor_scan(v_t[:, :], beta_t[:, :],
                                                     u_t[:, :], 0.0,
                                                     OP.mult, OP.add)
                        h_t = sc.tile([128, L], BF16, tag="h")
                        nc.vector.tensor_tensor_scan(h_t[:, :], a_t[:, :],
                                                     v_t[:, :], 0.0,
                                                     OP.mult, OP.add)
                        yterm = sc.tile([128, L], BF16, tag="yt")
                        nc.vector.tensor_tensor(out=yterm[:, :], in0=h_t[:, :],
                                                in1=cm_rep[:, :], op=OP.mult)
                        for t in range(NT):
                            nc.tensor.matmul(y_ps[m][:, t * 512:(t + 1) * 512],
                                             ident_t[:, :],
                                             yterm[:, t * 512:(t + 1) * 512],
                                             start=(n == 0), stop=(n == N_ST - 1))

                # ---- y + D*xc, gate (inside psY scope, t-chunked) ----
                g_t = res.tile([128, NE, L], F32R, tag="g")
                for m in range(NE):
                    for t in range(NT):
                        tsl = slice(t * 512, (t + 1) * 512)
                        yd = sc.tile([128, 512], F32, tag="yd", bufs=3)
                        nc.vector.scalar_tensor_tensor(
                            yd[:, :], ebl(xc_t, m).bitcast(F32)[:, tsl],
                            dcol_t[:, m, :],
                            y_ps[m][:, tsl], OP.mult, OP.add)
                        nc.vector.tensor_tensor(out=ebl(g_t, m)[:, tsl],
                                                in0=yd[:, :],
                                                in1=ebl(zs_t, m)[:, tsl],
                                                op=OP.mult)

            # =================== PHASE C: out_proj ===================
            with (
                tc.tile_pool(name="oc", bufs=4) as oc,
                tc.tile_pool(name="psC", bufs=4, space="PSUM") as psC,
            ):
                for mo in range(8):
                    for t in range(NT):
                        ps = psC.tile([128, 512], F32, tag="pC")
                        for m in range(NE):
                            nc.tensor.matmul(
                                ps[:],
                                wout_t[:, m, mo * 128:(mo + 1) * 128],
                                ebl(g_t, m)[:, t * 512:(t + 1) * 512],
                                start=(m == 0), stop=(m == NE - 1))
                        ot = oc.tile([128, 512], F32, tag="ot")
                        if (mo + t) % 2 == 0:
                            nc.scalar.copy(ot[:, :], ps[:])
                        else:
                            nc.vector.tensor_copy(ot[:, :], ps[:])
                        nc.sync.dma_start(
                            out=out_pT[mo * 128:(mo + 1) * 128,
                                       t * 512:(t + 1) * 512],
                            in_=ot[:, :])

    _split_ctrl_waits(nc)
    return nc


def _get_program():
    if "nc" not in _CACHE:
        _CACHE["nc"] = _build_program()
    return _CACHE["nc"]


def _in_maps(x, W_in, conv_w, conv_b, W_x, W_dt, b_dt, A_log, D, W_out):
    x = np.asarray(x, np.float32)
    xT = np.ascontiguousarray(x[0].T)                       # (1024, 2048)
    A = -np.exp(np.asarray(A_log, np.float32))              # (ED, N)
    ident = np.eye(128, dtype=ml_dtypes.bfloat16)
    W_in = np.asarray(W_in, np.float32)

    in_maps = []
    for j in range(N_CORES):
        sl = slice(j * E, (j + 1) * E)
        in_maps.append({
            "xT": xT,
            "wxcT": np.ascontiguousarray(W_in[sl, :].T),
            "wzT": np.ascontiguousarray(W_in[ED + j * E:ED + (j + 1) * E, :].T),
            "convw": np.ascontiguousarray(np.asarray(conv_w, np.float32)[sl]),
            "convb": np.ascontiguousarray(np.asarray(conv_b, np.float32)[sl])[:, None],
            "wxT": np.ascontiguousarray(np.asarray(W_x, np.float32)[:, sl].T),
            "wdtT": np.ascontiguousarray(np.asarray(W_dt, np.float32)[sl, :].T).astype(ml_dtypes.bfloat16),
            "bdt": np.ascontiguousarray(np.asarray(b_dt, np.float32)[sl])[:, None],
            "acols": np.ascontiguousarray(A[sl, :]),
            "dcol": np.ascontiguousarray(np.asarray(D, np.float32)[sl])[:, None],
            "woutT": np.ascontiguousarray(np.asarray(W_out, np.float32)[:, sl].T),
            "ident": ident,
        })
    return in_maps


def kernel(x, W_in, conv_w, conv_b, W_x, W_dt, b_dt, A_log, D, W_out):
    from concourse.bass_utils import run_bass_kernel_spmd

    nc = _get_program()
    in_maps = _in_maps(x, W_in, conv_w, conv_b, W_x, W_dt, b_dt, A_log, D, W_out)
    res = run_bass_kernel_spmd(nc, in_maps, list(range(N_CORES)))
    out_T = np.zeros((D_MODEL, L), np.float64)
    for j in range(N_CORES):
        out_T += res.results[j]["out_pT"]
    return out_T.T[None, :, :].astype(np.float32)
